# revision 4
# baseline (speedup 1.0000x reference)
"""Location-aware attention (qlen=1) on 8 TRN2 NeuronCores, data-parallel over batch.

Per-core plan (4 batches each):
  z[a, l]  = sum_d Wk[a, d] key[l, d]          f32r matmuls, keyT host-transposed
  th[a, l] = tanh(z + qproj[a] + bk[a])        ScalarE, fused per-partition bias, bf16 out
  e[l]     = sum_a v[a] th[a, l]               bf16 matmuls, M=125 chunks -> e on partitions
  aw       = softmax(e)                        exact softmax (no max-sub; |e| <= 22.6)
  cv       = sum_l aw[l] value[l, :]           f32r matmuls
aw_prev is zero and mask is all-ones in this problem instance (conv path contributes
exactly 0); a full numpy fallback handles any other case.
"""

import os

import numpy as np
import ml_dtypes
import jax

try:
    jax.config.update("jax_compilation_cache_dir", "/tmp/jax_neff_cache")
    jax.config.update("jax_persistent_cache_min_compile_time_secs", 10.0)
    jax.config.update("jax_persistent_cache_min_entry_size_bytes", 0)
except Exception:
    pass


def _install_neff_cache():
    """Skip the multi-minute walrus compile when the BIR is unchanged."""
    import hashlib
    import shutil

    import concourse.bass_utils as _bu
    import concourse.bass2jax as _b2j

    if getattr(_bu, "_neff_cache_installed", False):
        return
    orig = _bu.compile_bir_kernel

    def cached(bir_json, tmpdir, neff_name="file.neff"):
        h = hashlib.sha256(
            bir_json if isinstance(bir_json, bytes) else bir_json.encode()
        ).hexdigest()[:32]
        cdir = os.environ.get("NEFF_CACHE_DIR", "/tmp/neff_cache")
        os.makedirs(cdir, exist_ok=True)
        cpath = os.path.join(cdir, h + ".neff")
        if os.path.exists(cpath):
            dst = os.path.join(tmpdir, neff_name)
            shutil.copy(cpath, dst)
            return dst
        p = orig(bir_json, tmpdir, neff_name)
        try:
            shutil.copy(p, cpath)
        except Exception:
            pass
        return p

    _bu.compile_bir_kernel = cached
    _b2j.compile_bir_kernel = cached
    _bu._neff_cache_installed = True


_install_neff_cache()

import concourse.bacc as bacc
import concourse.mybir as mybir
from concourse.tile import TileContext
from concourse.bass_utils import run_bass_kernel_spmd

F32 = mybir.dt.float32
F32R = mybir.dt.float32r
BF16 = mybir.dt.bfloat16

B, KLEN, KDIM, QDIM, ADIM, VDIM = 32, 2000, 512, 512, 512, 512
CONV_CH, CONV_K = 10, 201
NCORES = 8
BPC = B // NCORES        # batches per core
NKC = KDIM // 128        # contraction chunks
NAC = ADIM // 128        # adim chunks
LT = 500                 # klen tile (one PSUM bank of f32)
NLT = KLEN // LT         # 4
LP = 125                 # klen partition chunk (16 x 125 = 2000, no padding)
NU = LT // LP            # 4 chunks per L tile
NLP = KLEN // LP         # 16

_STATE = {}
LAST_RESULTS = None


def _round_f32r(x: np.ndarray) -> np.ndarray:
    """Round f32 to the fp32r grid (11-bit mantissa, RNE) the PE uses."""
    u = np.ascontiguousarray(x, dtype=np.float32).view(np.uint32)
    r = (u + np.uint32(0x7FF) + ((u >> np.uint32(12)) & np.uint32(1))) & np.uint32(
        0xFFFFF000
    )
    return r.view(np.float32)


def _build():
    nc = bacc.Bacc("TRN2", target_bir_lowering=False)

    keyT = nc.declare_dram_parameter("keyT", [BPC, KDIM, KLEN], F32R, isOutput=False)
    val = nc.declare_dram_parameter("val", [BPC, KLEN, VDIM], F32R, isOutput=False)
    wkT = nc.declare_dram_parameter("wkT", [KDIM, ADIM], F32R, isOutput=False)
    wqT = nc.declare_dram_parameter("wqT", [QDIM, ADIM], F32R, isOutput=False)
    qT = nc.declare_dram_parameter("qT", [QDIM, BPC], F32R, isOutput=False)
    bk = nc.declare_dram_parameter("bk", [128, NAC], F32, isOutput=False)
    vv = nc.declare_dram_parameter("vv", [128, NAC], BF16, isOutput=False)
    ident = nc.declare_dram_parameter("ident", [128, 128], F32, isOutput=False)
    ones = nc.declare_dram_parameter("ones", [128, 128], F32, isOutput=False)
    cv_out = nc.declare_dram_parameter("cv_out", [BPC, VDIM], F32, isOutput=True)
    aw_out = nc.declare_dram_parameter("aw_out", [BPC, KLEN], F32, isOutput=True)

    with TileContext(nc) as tc:
        with (
            tc.tile_pool(name="consts", bufs=1) as consts,
            tc.tile_pool(name="keyp", bufs=3) as keyp,
            tc.tile_pool(name="valp", bufs=8) as valp,
            tc.tile_pool(name="thp", bufs=2) as thp,
            tc.tile_pool(name="smalls", bufs=2) as smalls,
            tc.tile_pool(name="zp", bufs=4, space="PSUM") as zp,
            tc.tile_pool(name="ep", bufs=2, space="PSUM") as ep,
            tc.tile_pool(name="sp", bufs=2, space="PSUM") as sp,
        ):
            wk_sb = consts.tile([128, NKC, ADIM], F32R)
            wq_sb = consts.tile([128, NKC, ADIM], F32R)
            q_sb = consts.tile([128, NKC, BPC], F32R)
            bk_sb = consts.tile([128, NAC], F32)
            v_sb = consts.tile([128, NAC], BF16)
            id_sb = consts.tile([128, 128], F32)
            on_sb = consts.tile([128, 128], F32)
            nc.sync.dma_start(out=wk_sb, in_=wkT[:].rearrange("(c p) a -> p c a", p=128))
            nc.sync.dma_start(out=wq_sb, in_=wqT[:].rearrange("(c p) a -> p c a", p=128))
            nc.sync.dma_start(out=q_sb, in_=qT[:].rearrange("(c p) b -> p c b", p=128))
            nc.sync.dma_start(out=bk_sb, in_=bk[:])
            nc.sync.dma_start(out=v_sb, in_=vv[:])
            nc.sync.dma_start(out=id_sb, in_=ident[:])
            nc.sync.dma_start(out=on_sb, in_=ones[:])

            # qproj for all 4 batches, then qb = qproj + bk (per-partition bias)
            qp_ps = sp.tile([128, NAC * BPC], F32, tag="sm")
            for ac in range(NAC):
                for kc in range(NKC):
                    nc.tensor.matmul(
                        qp_ps[:, ac * BPC:(ac + 1) * BPC],
                        wq_sb[:, kc, ac * 128:(ac + 1) * 128],
                        q_sb[:, kc, :],
                        start=(kc == 0),
                        stop=(kc == NKC - 1),
                    )
            qb_sb = consts.tile([128, NAC * BPC], F32)
            for ac in range(NAC):
                nc.scalar.activation(
                    qb_sb[:, ac * BPC:(ac + 1) * BPC],
                    qp_ps[:, ac * BPC:(ac + 1) * BPC],
                    mybir.ActivationFunctionType.Identity,
                    bias=bk_sb[:, ac:ac + 1],
                )

            for b in range(BPC):
                key_tiles = []
                val_tiles = []
                for T in range(NLT):
                    kt = keyp.tile([128, NKC, LT], F32R, tag="key")
                    nc.sync.dma_start(
                        out=kt,
                        in_=keyT[b, :, T * LT:(T + 1) * LT].rearrange(
                            "(c p) l -> p c l", p=128
                        ),
                    )
                    key_tiles.append(kt)
                for T in range(NLT):
                    vt = valp.tile([LP, NU, VDIM], F32R, tag="val")
                    nc.sync.dma_start(
                        out=vt,
                        in_=val[b, T * LT:(T + 1) * LT, :].rearrange(
                            "(u p) v -> p u v", p=LP
                        ),
                    )
                    val_tiles.append(vt)

                e_ps = ep.tile([LP, NLP], F32)
                for T in range(NLT):
                    th_sb = thp.tile([128, NAC, LT], BF16, tag="th")
                    for ac in range(NAC):
                        z_ps = zp.tile([128, LT], F32, tag="z")
                        for kc in range(NKC):
                            nc.tensor.matmul(
                                z_ps,
                                wk_sb[:, kc, ac * 128:(ac + 1) * 128],
                                key_tiles[T][:, kc, :],
                                start=(kc == 0),
                                stop=(kc == NKC - 1),
                            )
                        nc.scalar.activation(
                            th_sb[:, ac, :],
                            z_ps,
                            mybir.ActivationFunctionType.Tanh,
                            bias=qb_sb[:, ac * BPC + b:ac * BPC + b + 1],
                        )
                    for u in range(NU):
                        col = T * NU + u
                        for ac in range(NAC):
                            nc.tensor.matmul(
                                e_ps[:, col:col + 1],
                                th_sb[:, ac, u * LP:(u + 1) * LP],
                                v_sb[:, ac:ac + 1],
                                start=(ac == 0),
                                stop=(ac == NAC - 1),
                            )

                # softmax over all 2000 positions (exact: no max subtraction,
                # |e| <= sum|v| * 1 <= 23 so exp never overflows)
                exp_sb = smalls.tile([LP, NLP], F32, tag="exp")
                nc.scalar.activation(exp_sb, e_ps, mybir.ActivationFunctionType.Exp)
                red_sb = smalls.tile([LP, 1], F32, tag="red")
                nc.vector.reduce_sum(red_sb, exp_sb, axis=mybir.AxisListType.X)
                s_ps = sp.tile([128, 1], F32, tag="sm")
                nc.tensor.matmul(s_ps, on_sb[:LP, :], red_sb, start=True, stop=True)
                inv_sb = smalls.tile([128, 1], F32, tag="inv")
                nc.vector.reciprocal(inv_sb, s_ps)
                aw_sb = smalls.tile([LP, NLP], F32, tag="aw")
                nc.vector.tensor_scalar_mul(aw_sb, exp_sb, inv_sb[:LP, :])
                aw_r = smalls.tile([LP, NLP], F32R, tag="awr")
                nc.vector.tensor_copy(aw_r, aw_sb)

                # cv = sum_l aw[l] * value[l, :]
                cv_ps = sp.tile([1, VDIM], F32, tag="sm")
                for t in range(NLP):
                    nc.tensor.matmul(
                        cv_ps,
                        aw_r[:, t:t + 1],
                        val_tiles[t // NU][:, t % NU, :],
                        start=(t == 0),
                        stop=(t == NLP - 1),
                    )
                cv_sb = smalls.tile([1, VDIM], F32, tag="cv")
                nc.vector.tensor_copy(cv_sb, cv_ps)
                nc.sync.dma_start(out=cv_out[b:b + 1, :], in_=cv_sb)

                # aw transposed to [16, 125] so the DRAM write is contiguous
                awt_ps = sp.tile([NLP, LP], F32, tag="sm")
                nc.tensor.transpose(awt_ps, aw_sb, id_sb[:LP, :LP])
                awt_sb = smalls.tile([NLP, LP], F32, tag="awt")
                nc.vector.tensor_copy(awt_sb, awt_ps)
                nc.sync.dma_start(
                    out=aw_out[b].rearrange("(t p) -> t p", p=LP), in_=awt_sb
                )
    nc.compile()
    return nc


def _get_nc():
    if "nc" not in _STATE:
        _STATE["nc"] = _build()
    return _STATE["nc"]


def _numpy_reference(key, value, query, aw_prev, mask, Wk, bk, Wq, conv_w, Wconv, v):
    """Exact fallback, used only if aw_prev/mask deviate from the spec fills."""
    from numpy.lib.stride_tricks import sliding_window_view

    key = np.asarray(key, np.float32)
    value = np.asarray(value, np.float32)
    query = np.asarray(query, np.float32)
    aw_prev = np.asarray(aw_prev, np.float32)
    kproj = np.einsum("bkd,ad->bka", key, Wk) + bk
    qproj = np.einsum("bqd,ad->bqa", query, Wq)
    pad = (CONV_K - 1) // 2
    aw_pad = np.pad(aw_prev, ((0, 0), (0, 0), (pad, pad)))
    win = sliding_window_view(aw_pad, CONV_K, axis=-1)  # [B, 1, KLEN, CONV_K]
    conv_feat = np.einsum("bqkj,cqj->bck", win, conv_w)
    conv_proj = np.einsum("bck,ac->bka", conv_feat, Wconv)
    tmp = np.tanh(kproj[:, None] + qproj[:, :, None] + conv_proj[:, None])
    e = np.einsum("bqka,a->bqk", tmp, v).astype(np.float32)
    e = np.where(np.asarray(mask) == 0, np.float32(np.finfo(np.float32).min), e)
    e = e - e.max(axis=-1, keepdims=True)
    ex = np.exp(e)
    aw = ex / ex.sum(axis=-1, keepdims=True)
    cv = np.einsum("bqk,bkv->bqv", aw, value).astype(np.float32)
    return cv, aw[:, None].astype(np.float32)


def kernel(key, value, query, aw_prev, mask, Wk, bk, Wq, conv_w, Wconv, v):
    global LAST_RESULTS
    key = np.asarray(key, np.float32)
    value = np.asarray(value, np.float32)
    query = np.asarray(query, np.float32)
    if np.any(np.asarray(aw_prev) != 0) or np.any(np.asarray(mask) == 0):
        return _numpy_reference(
            key, value, query, aw_prev, mask, Wk, bk, Wq, conv_w, Wconv, v
        )

    keyT = _round_f32r(np.ascontiguousarray(key.transpose(0, 2, 1)))
    val_r = _round_f32r(value)
    wkT_h = _round_f32r(np.asarray(Wk, np.float32).T)        # [KDIM, ADIM]
    wqT_h = _round_f32r(np.asarray(Wq, np.float32).T)        # [QDIM, ADIM]
    qT_h = _round_f32r(query[:, 0, :].T)                     # [QDIM, B]
    bk_h = np.asarray(bk, np.float32).reshape(NAC, 128).T.copy()
    vv_h = np.asarray(v, np.float32).reshape(NAC, 128).T.astype(ml_dtypes.bfloat16)
    id_h = np.eye(128, dtype=np.float32)
    on_h = np.ones((128, 128), dtype=np.float32)

    in_maps = []
    for c in range(NCORES):
        s = slice(c * BPC, (c + 1) * BPC)
        in_maps.append(
            dict(
                keyT=keyT[s],
                val=val_r[s],
                wkT=wkT_h,
                wqT=wqT_h,
                qT=np.ascontiguousarray(qT_h[:, s]),
                bk=bk_h,
                vv=vv_h,
                ident=id_h,
                ones=on_h,
            )
        )

    nc = _get_nc()
    trace = os.environ.get("KERNEL_TRACE") == "1"
    res = run_bass_kernel_spmd(
        nc,
        in_maps,
        core_ids=list(range(NCORES)),
        trace=trace,
        tmpdir=os.environ.get("KERNEL_TRACE_DIR") or None,
    )
    LAST_RESULTS = res

    cv = np.concatenate([res.results[c]["cv_out"] for c in range(NCORES)], axis=0)
    aw = np.concatenate([res.results[c]["aw_out"] for c in range(NCORES)], axis=0)
    return (
        cv.reshape(B, 1, VDIM),
        aw.reshape(B, 1, 1, KLEN),
    )
